# revision 22
# baseline (speedup 1.0000x reference)
"""Trainium2 Bass kernel for a 2-layer LSTM (B=4096, T=168, D=16, H=96) + FC head.

Strategy: pure data parallel over 8 NeuronCores (512 batch rows each).
Per core, gate-major layout: the recurrent matmul computes gates.T
[gate, batch] with weights stationary on the PE, so hidden state h stays in
[feature, batch] layout across steps and never needs a transpose.

v2 changes vs baseline:
- fp16 everywhere off-PSUM: weights, x, h, gate activations, and the c state
  are float16. Matmuls run fp16 (1 cy/row, same as fp32r) with fp32 PSUM.
  All DVE tensor_tensor ops become 2x-mode (2-byte operands), halving DVE
  busy time. x DMA bytes are halved.
- Gate order [f, i, o, g] with the g-gate weight columns pre-scaled by 2 so
  tanh(g) = 2*sigmoid(2g) - 1: ONE sigmoid op covers all four gate blocks
  [128,2048]; tanh(g) is recovered with a cheap DVE tensor_scalar.
- Layer-1 lags layer-0 by TWO slots (h1 ring of 3), fully decoupling the two
  recurrence chains so L1 work fills L0 chain-latency gaps without gluing.
- Engine queue order gives the L0 chain priority (mm0 before mm1, sigma0 /
  tanh_c0 before L1's ACT ops).
"""

import numpy as np

import concourse.bass as bass
import concourse.bacc as bacc
import concourse.tile as tile
from concourse import mybir
from concourse.bass_utils import run_bass_kernel_spmd

B, T, D, H = 4096, 168, 16, 96
NCORES = 8
BS = B // NCORES  # 512 batch rows per core
F32 = mybir.dt.float32
F16 = mybir.dt.float16
SIG = mybir.ActivationFunctionType.Sigmoid
TANH = mybir.ActivationFunctionType.Tanh

# gate row slices in torch order (i, f, g, o) -> our tile order [f, i, g, o]
# tile col block b holds gate _GATE_SRC[b] of the torch layout.
_GATE_SRC = [(96, 192), (0, 96), (192, 288), (288, 384)]  # f, i, g, o

TRACE = False
LAST = {}
T_RUN = T
MM_DT = F16


def _prep_weights(Wih0, Whh0, bih0, bhh0, Wih1, Whh1, bih1, bhh1, Wfc, bfc):
    w0 = np.zeros((113, 512), np.float32)  # rows: h(96), x(16), const(1)
    w1a = np.zeros((96, 512), np.float32)  # rows: h1(96)
    w1b = np.zeros((97, 512), np.float32)  # rows: h2(96), const(1)
    for gi, (r0, r1) in enumerate(_GATE_SRC):
        sc = 2.0 if gi == 2 else 1.0  # g-gate pre-scaled: tanh(g)=2*sig(2g)-1
        c0, c1 = 128 * gi, 128 * gi + 96
        w0[0:96, c0:c1] = sc * Whh0[r0:r1, :].T
        w0[96:112, c0:c1] = sc * Wih0[r0:r1, :].T
        w0[112, c0:c1] = sc * (bih0[r0:r1] + bhh0[r0:r1])
        w1a[:, c0:c1] = sc * Wih1[r0:r1, :].T
        w1b[0:96, c0:c1] = sc * Whh1[r0:r1, :].T
        w1b[96, c0:c1] = sc * (bih1[r0:r1] + bhh1[r0:r1])
    wfc = np.zeros((97, 1), np.float32)
    wfc[0:96, 0] = Wfc[0, :]
    wfc[96, 0] = bfc[0]
    return w0, w1a, w1b, wfc


def _build_nc():
    nc = bacc.Bacc("TRN2", target_bir_lowering=False)
    xs_d = nc.dram_tensor("xs", [T, D + 1, BS], MM_DT, kind="ExternalInput")
    w0_d = nc.dram_tensor("w0", [113, 512], MM_DT, kind="ExternalInput")
    w1a_d = nc.dram_tensor("w1a", [96, 512], MM_DT, kind="ExternalInput")
    w1b_d = nc.dram_tensor("w1b", [97, 512], MM_DT, kind="ExternalInput")
    wfc_d = nc.dram_tensor("wfc", [97, 1], MM_DT, kind="ExternalInput")
    y_d = nc.dram_tensor("y", [1, BS], F32, kind="ExternalOutput")

    with tile.TileContext(nc) as tc:
        with (
            tc.tile_pool(name="persist", bufs=1) as P,
            tc.tile_pool(name="sig", bufs=2) as SIGP,
            tc.tile_pool(name="tgp", bufs=2) as TGP,
            tc.tile_pool(name="tcp", bufs=2) as TCP,
            tc.tile_pool(name="qp", bufs=2) as QPP,
            tc.tile_pool(name="ps", bufs=1, space="PSUM") as PSP,
        ):
            # DMA into staging tiles, then DVE-copy into the tiles matmuls
            # read, so matmul waits only involve {DVE, ACT} sems.
            w0_g = P.tile([113, 512], MM_DT, tag="w0_g")
            w1a_g = P.tile([96, 512], MM_DT, tag="w1a_g")
            w1b_g = P.tile([97, 512], MM_DT, tag="w1b_g")
            wfc_g = P.tile([97, 1], MM_DT, tag="wfc_g")
            nc.gpsimd.dma_start(out=w0_g[:, :], in_=w0_d[:, :])
            nc.gpsimd.dma_start(out=w1a_g[:, :], in_=w1a_d[:, :])
            nc.gpsimd.dma_start(out=w1b_g[:, :], in_=w1b_d[:, :])
            nc.gpsimd.dma_start(out=wfc_g[:, :], in_=wfc_d[:, :])
            w0_s = P.tile([113, 512], MM_DT, tag="w0")
            w1a_s = P.tile([96, 512], MM_DT, tag="w1a")
            w1b_s = P.tile([97, 512], MM_DT, tag="w1b")
            wfc_s = P.tile([97, 1], MM_DT, tag="wfc")
            nc.vector.tensor_copy(w0_s[:, :], w0_g[:, :])
            nc.vector.tensor_copy(w1a_s[:, :], w1a_g[:, :])
            nc.vector.tensor_copy(w1b_s[:, :], w1b_g[:, :])
            nc.vector.tensor_copy(wfc_s[:, :], wfc_g[:, :])

            # rhs0 ring of 3: [h1(0:96); x_t(96:112); 1.0(112)]
            # rhs1 ring of 2: [h2(0:96); 1.0(96)]
            rhs0 = [P.tile([113, BS], MM_DT, tag=f"rhs0_{i}", name=f"rhs0_{i}") for i in range(3)]
            rhs1 = [P.tile([97, BS], MM_DT, tag=f"rhs1_{i}", name=f"rhs1_{i}") for i in range(2)]
            c0 = P.tile([96, BS], F16, tag="c0")
            c1 = P.tile([96, BS], F16, tag="c1")
            for i in range(3):
                nc.vector.memset(rhs0[i][:, :], 0.0)
            for i in range(2):
                nc.vector.memset(rhs1[i][:, :], 0.0)
                nc.vector.memset(rhs1[i][96:97, :], 1.0)
            nc.vector.memset(c0[:, :], 0.0)
            nc.vector.memset(c1[:, :], 0.0)

            nc.gpsimd.dma_start(out=rhs0[0][96:113, :], in_=xs_d[0, :, :])

            def l0_mm(s):
                # layer-0 matmuls for step s: consumes rhs0[s%3]
                g0 = PSP.tile([128, 2048], F32, tag="g0", name="g0")
                for g in range(4):
                    nc.tensor.matmul(
                        out=g0[:, 512 * g : 512 * (g + 1)],
                        lhsT=w0_s[:, 128 * g : 128 * (g + 1)],
                        rhs=rhs0[s % 3][:, :],
                        start=True,
                        stop=True,
                    )
                return g0

            def l1_mm_a(u):
                # w1a (h1) accumulation passes for L1 step u, emitted at
                # slot u+2: h1_u lives in rhs0[(u+1)%3][0:96], ready since
                # slot u (two slots old -> never blocks the PE queue).
                g1 = PSP.tile([128, 2048], F32, tag="g1", name="g1")
                for g in range(4):
                    nc.tensor.matmul(
                        out=g1[:, 512 * g : 512 * (g + 1)],
                        lhsT=w1a_s[:, 128 * g : 128 * (g + 1)],
                        rhs=rhs0[(u + 1) % 3][0:96, :],
                        start=True,
                        stop=False,
                    )
                return g1

            def l1_mm_b(u, g1):
                # w1b (h2) accumulation passes for L1 step u, emitted at
                # slot u+3: h2_{u-1} was written at slot u+2, so these never
                # block the in-order PE queue (and so never delay mm0).
                for g in range(4):
                    nc.tensor.matmul(
                        out=g1[:, 512 * g : 512 * (g + 1)],
                        lhsT=w1b_s[:, 128 * g : 128 * (g + 1)],
                        rhs=rhs1[u % 2][0:97, :],
                        start=False,
                        stop=True,
                    )
                return g1

            MULT = mybir.AluOpType.mult
            SUB = mybir.AluOpType.subtract

            def l0_elem(s, g0):
                # gate cols: f 0:512, i 512:1024, 2g 1024:1536, o 1536:2048
                # sigma split [f,i,g | o]: the c-critical part starts after
                # only 3 matmuls. The o-gate preacts are copied out of PSUM
                # by the (idle) gpsimd engine so the g0 WAR clears early and
                # the o-sigmoid can run late on ACT without blocking mm0.
                sig0 = SIGP.tile([128, 2048], F16, tag="sig0", name="sig0")
                tg0 = TGP.tile([96, BS], F16, tag="tg0", name="tg0")
                nc.scalar.activation(out=sig0[:, 0:1536], in_=g0[:, 0:1536], func=SIG)
                nc.scalar.activation(out=sig0[:, 1536:2048], in_=g0[:, 1536:2048], func=SIG)
                q0 = QPP.tile([96, BS], F16, tag="q0", name="q0")
                p0 = QPP.tile([96, BS], F16, tag="p0", name="p0")
                nc.vector.tensor_mul(q0[:, :], sig0[0:96, 0:512], c0[:, :])
                # tanh(g) = 2*sig(2g) - 1
                nc.vector.tensor_scalar(
                    tg0[:, :], sig0[0:96, 1024:1536], 2.0, 1.0, MULT, SUB
                )
                nc.vector.tensor_mul(p0[:, :], sig0[0:96, 512:1024], tg0[:, :])
                nc.vector.tensor_add(c0[:, :], q0[:, :], p0[:, :])
                tc0 = TCP.tile([96, BS], F16, tag="tc0", name="tc0")
                nc.scalar.activation(out=tc0[:, :], in_=c0[:, :], func=TANH)
                nc.vector.tensor_mul(
                    rhs0[(s + 1) % 3][0:96, :], sig0[0:96, 1536:2048], tc0[:, :]
                )

            def l1_elem(u, g1):
                sig1 = SIGP.tile([128, 2048], F16, tag="sig1", name="sig1")
                tg1 = TGP.tile([96, BS], F16, tag="tg1", name="tg1")
                nc.scalar.activation(out=sig1[:, 0:1536], in_=g1[:, 0:1536], func=SIG)
                nc.scalar.activation(out=sig1[:, 1536:2048], in_=g1[:, 1536:2048], func=SIG)
                q1 = QPP.tile([96, BS], F16, tag="q1", name="q1")
                p1 = QPP.tile([96, BS], F16, tag="p1", name="p1")
                nc.vector.tensor_mul(q1[:, :], sig1[0:96, 0:512], c1[:, :])
                nc.vector.tensor_scalar(
                    tg1[:, :], sig1[0:96, 1024:1536], 2.0, 1.0, MULT, SUB
                )
                nc.vector.tensor_mul(p1[:, :], sig1[0:96, 512:1024], tg1[:, :])
                nc.vector.tensor_add(c1[:, :], q1[:, :], p1[:, :])
                tc1 = TCP.tile([96, BS], F16, tag="tc1", name="tc1")
                nc.scalar.activation(out=tc1[:, :], in_=c1[:, :], func=TANH)
                nc.vector.tensor_mul(
                    rhs1[(u + 1) % 2][0:96, :], sig1[0:96, 1536:2048], tc1[:, :]
                )

            # Slot s emission: mm0(s); w1b completing L1 step s-2's gates
            # (inputs a slot old -> never blocks the PE queue); L0
            # elementwise; L1 step s-2 elementwise; w1a starting L1 step
            # s-1's gates (after the sigma reads of step s-2 above, for
            # correct PSUM WAR order).
            # L1 lags L0 by 3 slots. Per slot s: mm0(s); w1b completing L1
            # step s-3 (its h2 input is a slot old); L0 elementwise; L1
            # step s-3 elementwise; w1a opening L1 step s-2 (h1 input two
            # slots old). Only mm0 — the true pacer — ever blocks the
            # in-order PE queue.
            g1_a_done = None
            for s in range(T_RUN + 3):
                if s + 1 < T_RUN:
                    nc.gpsimd.dma_start(
                        out=rhs0[(s + 1) % 3][96:113, :], in_=xs_d[s + 1, :, :]
                    )
                g0 = l0_mm(s) if s < T_RUN else None
                g1_full = l1_mm_b(s - 3, g1_a_done) if g1_a_done is not None else None
                if g0 is not None:
                    l0_elem(s, g0)
                if g1_full is not None:
                    l1_elem(s - 3, g1_full)
                g1_a_done = l1_mm_a(s - 2) if 2 <= s <= T_RUN + 1 else None

            # ---- FC head on h2 at t = T-1 (written to rhs1[T%2]) ----
            fc_ps = PSP.tile([1, 512], F32, tag="g0")
            nc.tensor.matmul(
                out=fc_ps[:, :],
                lhsT=wfc_s[:, :],
                rhs=rhs1[T_RUN % 2][0:97, :],
                start=True,
                stop=True,
            )
            y_s = P.tile([1, 512], F32, tag="y")
            nc.vector.tensor_copy(y_s[:, :], fc_ps[:, :])
            nc.gpsimd.dma_start(out=y_d[:, :], in_=y_s[:, :])
    nc.compile()
    return nc



def _ensure_ntff_hook():
    """Provide antenv.axon_hooks (absent in this image) so trace=True works."""
    import sys, types, ctypes, contextlib
    try:
        import antenv.axon_hooks  # noqa: F401
        return
    except ImportError:
        pass
    mod = types.ModuleType("antenv.axon_hooks")
    holder = {}
    mod.set_axon_ntff_profile_hook = lambda h: holder.__setitem__("h", h)
    mod.get_axon_ntff_profile_hook = lambda: holder.get("h")
    sys.modules["antenv.axon_hooks"] = mod
    lib = ctypes.CDLL("/opt/axon/libaxon_pjrt.so")
    if not hasattr(lib, "axon_start_nrt_profile"):
        return
    lib.axon_start_nrt_profile.argtypes = [
        ctypes.POINTER(ctypes.c_int64), ctypes.c_size_t]
    lib.axon_start_nrt_profile.restype = ctypes.c_int64
    lib.axon_stop_nrt_profile.argtypes = [ctypes.c_char_p]
    lib.axon_stop_nrt_profile.restype = ctypes.c_int64

    @contextlib.contextmanager
    def _hook(output_dir, device_ids):
        import jax
        jax.devices()
        if device_ids:
            ids = (ctypes.c_int64 * len(device_ids))(*device_ids)
            rc = lib.axon_start_nrt_profile(ids, len(device_ids))
        else:
            rc = lib.axon_start_nrt_profile(None, 0)
        if rc != 0:
            raise RuntimeError(f"axon_start_nrt_profile rc={rc}")
        try:
            yield
        finally:
            n = lib.axon_stop_nrt_profile(str(output_dir).encode())
            print(f"ntff profile: {n} file(s) written to {output_dir}")

    mod.set_axon_ntff_profile_hook(_hook)


def _patch_upload():
    """Skip artifact upload to remote storage (no share in this container)."""
    import concourse.bass_utils as bu
    bu.upload_artifacts = lambda tmpdir: tmpdir


_NC = None


def kernel(x, Wih0, Whh0, bih0, bhh0, Wih1, Whh1, bih1, bhh1, Wfc, bfc):
    global _NC
    arrs = [np.asarray(a, np.float32) for a in (
        x, Wih0, Whh0, bih0, bhh0, Wih1, Whh1, bih1, bhh1, Wfc, bfc)]
    x = arrs[0]
    w0, w1a, w1b, wfc = _prep_weights(*arrs[1:])
    w0 = w0.astype(np.float16)
    w1a = w1a.astype(np.float16)
    w1b = w1b.astype(np.float16)
    wfc = wfc.astype(np.float16)
    if _NC is None:
        _NC = _build_nc()
    in_maps = []
    for core in range(NCORES):
        xt = x[core * BS : (core + 1) * BS].transpose(1, 2, 0)  # [T, D, BS]
        xs = np.concatenate(
            [xt, np.ones((T, 1, BS), np.float32)], axis=1
        ).astype(np.float16)  # [T, D+1, BS] with const-1 row
        in_maps.append({"xs": xs, "w0": w0, "w1a": w1a, "w1b": w1b, "wfc": wfc})
    if TRACE:
        _ensure_ntff_hook()
        _patch_upload()
    import tempfile
    tdir = tempfile.mkdtemp(prefix="lstm_prof_") if TRACE else None
    res = run_bass_kernel_spmd(
        _NC, in_maps, core_ids=list(range(NCORES)), trace=TRACE, tmpdir=tdir
    )
    LAST["tmpdir"] = tdir
    LAST["exec_time_ns"] = res.exec_time_ns
    LAST["profile_json"] = res.profile_json
    y = np.concatenate([res.results[i]["y"][0] for i in range(NCORES)])
    return y.astype(np.float32)
